# revision 11
# baseline (speedup 1.0000x reference)
"""Trainium2 Bass kernel for nn_AffineCoupling_884763263531.

Reference computation (B=65536, D=512, H=2048, half=256):
    a = z[:, 0::2]; b = z[:, 1::2]
    s = relu(a @ W1_s + b1_s) @ W2_s + b2_s
    t = relu(a @ W1_t + b1_t) @ W2_t + b2_t
    z_out = interleave(a, b * exp(s) + t);  logdet = s.sum(1)

Strategy: data-parallel over batch across 8 cores (8192 rows each).
The host hands each core feature-major (transposed) activations so both MLP
layers contract along the partition dim with zero on-device transposes:
  at [256, 8192] fp16  = a^T          (matmul operand)
  bt [256, 8192] fp32  = b^T          (exact passthrough for the epilogue)
Device: L1 fp16 matmuls -> PSUM -> fused bias+relu evac to fp16 h;
L2 fp16 matmuls -> s,t; logdet via ones-vector matmul over s^T partitions;
epilogue b_out^T = b^T * exp(s^T) + t^T, all feature-major.
Host re-interleaves: z_out[:,0::2]=a (exact), z_out[:,1::2]=b_out^T.T.
"""

import sys

for _p in ("/opt/trn_rl_repo",):
    if _p not in sys.path:
        sys.path.append(_p)

import numpy as np

import concourse.bass as bass
import concourse.mybir as mybir
from concourse import bacc
from concourse import bass_utils
from concourse.tile import TileContext

NCORES = 8
B, D, H = 65536, 512, 2048
HALF = D // 2           # 256
R_FULL = B // NCORES    # 8192 rows per core
BLK = 512               # rows per block
HC = H // 128           # 16 hidden chunks
FC = HALF // 128        # 2 output-feature chunks

F32 = mybir.dt.float32
F16 = mybir.dt.float16
AF = mybir.ActivationFunctionType
ALU = mybir.AluOpType


def build(rows=R_FULL, blk=BLK):
    # last 512 rows as two 256-row blocks: halves the non-overlapped tail
    blocks = []
    r0 = 0
    while r0 < rows:
        bsz = blk if r0 + blk < rows or rows <= blk else blk // 2
        bsz = min(bsz, rows - r0)
        blocks.append((r0, bsz))
        r0 += bsz
    nc = bacc.Bacc("TRN2", debug=False, num_devices=NCORES)

    at = nc.dram_tensor("at", [HALF, rows], F16, kind="ExternalInput")
    bt = nc.dram_tensor("bt", [HALF, rows], F32, kind="ExternalInput")
    w1 = nc.dram_tensor("w1", [128, 2 * 2 * H], F16, kind="ExternalInput")
    w2 = nc.dram_tensor("w2", [128, 2 * HC * HALF], F16, kind="ExternalInput")
    b1 = nc.dram_tensor("b1", [128, 2 * HC], F32, kind="ExternalInput")
    b2 = nc.dram_tensor("b2", [128, 2 * FC], F32, kind="ExternalInput")
    bo = nc.dram_tensor("bo", [HALF, rows], F32, kind="ExternalOutput")
    ld = nc.dram_tensor("ld", [rows], F32, kind="ExternalOutput")

    with TileContext(nc) as tc:
        with tc.tile_pool(name="const", bufs=1) as const, \
             tc.tile_pool(name="ap_", bufs=4) as ap_, \
             tc.tile_pool(name="bp", bufs=4) as bp, \
             tc.tile_pool(name="hp", bufs=3) as hp, \
             tc.tile_pool(name="stp", bufs=2) as stp, \
             tc.tile_pool(name="ep", bufs=4) as ep, \
             tc.tile_pool(name="tmpp", bufs=4) as tmpp, \
             tc.tile_pool(name="bop", bufs=4) as bop, \
             tc.tile_pool(name="ldp", bufs=2) as ldp, \
             tc.tile_pool(name="ps_h", bufs=4, space="PSUM") as ps_h, \
             tc.tile_pool(name="ps_l2", bufs=3, space="PSUM") as ps_l2, \
             tc.tile_pool(name="ps_ld", bufs=1, space="PSUM") as ps_ld:

            w1_sb = const.tile([128, 2 * 2 * H], F16)
            w2_sb = const.tile([128, 2 * HC * HALF], F16)
            b1_sb = const.tile([128, 2 * HC], F32)
            b2_sb = const.tile([128, 2 * FC], F32)
            ones = const.tile([128, 1], F16)

            # weights ride the SWDGE queue (gpsimd) so block-0 activation
            # loads on the sync HWDGE queue aren't stuck behind 8 MB of weights
            for q in range(4):
                nc.gpsimd.dma_start(w1_sb[:, q * H:(q + 1) * H],
                                    w1[:, q * H:(q + 1) * H])
            nc.gpsimd.dma_start(b1_sb[:], b1[:])
            for q in range(4):
                nc.gpsimd.dma_start(w2_sb[:, q * 2048:(q + 1) * 2048],
                                    w2[:, q * 2048:(q + 1) * 2048])
            nc.gpsimd.dma_start(b2_sb[:], b2[:])
            nc.gpsimd.memset(ones[:], 1.0)

            for bi, (r0, bsz) in enumerate(blocks):
                ats = []
                bts = []
                for k in range(FC):
                    a_t = ap_.tile([128, bsz], F16, tag="at", name=f"at_{bi}_{k}")
                    nc.sync.dma_start(a_t[:], at[k * 128:(k + 1) * 128, r0:r0 + bsz])
                    ats.append(a_t)
                    b_t = bp.tile([128, bsz], F32, tag="bt", name=f"bt_{bi}_{k}")
                    nc.sync.dma_start(b_t[:], bt[k * 128:(k + 1) * 128, r0:r0 + bsz])
                    bts.append(b_t)

                # ---- layer 1 (both MLPs): hT[m] [p=h-chunk, free=(hc, row)] fp16
                hts = []
                for m in range(2):
                    ht = hp.tile([128, HC * bsz], F16, tag="h", name=f"h_{bi}_{m}")
                    hts.append(ht)
                    for hc in range(HC):
                        ph = ps_h.tile([128, bsz], F32, tag="ph", name=f"ph_{bi}_{m}_{hc}")
                        for k in range(FC):
                            nc.tensor.matmul(
                                ph[:],
                                w1_sb[:, ((m * 2 + k) * H + hc * 128):((m * 2 + k) * H + hc * 128 + 128)],
                                ats[k][:],
                                start=(k == 0), stop=(k == FC - 1))
                        dst = ht[:, hc * bsz:(hc + 1) * bsz]
                        bias_ap = b1_sb[:, m * HC + hc: m * HC + hc + 1]
                        if hc % 2 == 0:
                            nc.scalar.activation(dst, ph[:], AF.Relu, bias=bias_ap)
                        else:
                            nc.vector.tensor_scalar(
                                dst, ph[:], bias_ap, 0.0, op0=ALU.add, op1=ALU.max)

                # ---- layer 2: s (fp16, feeds exp+logdet), t (fp32)
                st_sb = []
                for m in range(2):
                    st = stp.tile([128, FC * bsz], F16 if m == 0 else F32,
                                  tag=("s" if m == 0 else "t"), name=f"st_{bi}_{m}")
                    st_sb.append(st)
                    for mc in range(FC):
                        pl2 = ps_l2.tile([128, bsz], F32, tag="l2", name=f"pl2_{bi}_{m}_{mc}")
                        for hc in range(HC):
                            nc.tensor.matmul(
                                pl2[:],
                                w2_sb[:, ((m * HC + hc) * HALF + mc * 128):((m * HC + hc) * HALF + mc * 128 + 128)],
                                hts[m][:, hc * bsz:(hc + 1) * bsz],
                                start=(hc == 0), stop=(hc == HC - 1))
                        dst = st[:, mc * bsz:(mc + 1) * bsz]
                        bias_ap = b2_sb[:, m * FC + mc: m * FC + mc + 1]
                        if m == 0:
                            nc.vector.tensor_scalar_add(dst, pl2[:], bias_ap)
                        else:
                            nc.scalar.activation(dst, pl2[:], AF.Identity, bias=bias_ap)

                # ---- logdet: ones^T @ s^T accumulated over both feature chunks
                pld = ps_ld.tile([1, bsz], F32, tag="ld", name=f"pld_{bi}")
                for mc in range(FC):
                    nc.tensor.matmul(pld[:], ones[:],
                                     st_sb[0][:, mc * bsz:(mc + 1) * bsz],
                                     start=(mc == 0), stop=(mc == FC - 1))
                ld_sb = ldp.tile([1, bsz], F32, tag="lds", name=f"lds_{bi}")
                nc.vector.tensor_copy(ld_sb[:], pld[:])
                nc.sync.dma_start(ld[r0:r0 + bsz], ld_sb[:])

                # ---- epilogue (feature-major): bo = bt * exp(s) + t
                for mc in range(FC):
                    sl = slice(mc * bsz, (mc + 1) * bsz)
                    ex = ep.tile([128, bsz], F32, tag="e", name=f"e_{bi}_{mc}")
                    nc.scalar.activation(ex[:], st_sb[0][:, sl], AF.Exp)
                    tmp = tmpp.tile([128, bsz], F32, tag="tm", name=f"tm_{bi}_{mc}")
                    nc.vector.tensor_mul(tmp[:], ex[:], bts[mc][:])
                    bo_sb = bop.tile([128, bsz], F32, tag="bo", name=f"bo_{bi}_{mc}")
                    nc.vector.tensor_add(bo_sb[:], tmp[:], st_sb[1][:, sl])
                    nc.sync.dma_start(bo[mc * 128:(mc + 1) * 128, r0:r0 + bsz], bo_sb[:])

    nc.compile()
    return nc


def prep_weights(W1_s, b1_s, W2_s, b2_s, W1_t, b1_t, W2_t, b2_t):
    def w1c(w):
        return w.reshape(2, 128, H).transpose(1, 0, 2)

    def w2c(w):
        return w.reshape(HC, 128, HALF).transpose(1, 0, 2)

    w1 = np.stack([w1c(W1_s), w1c(W1_t)], axis=1).reshape(128, 2 * 2 * H)
    w2 = np.stack([w2c(W2_s), w2c(W2_t)], axis=1).reshape(128, 2 * HC * HALF)
    b1 = np.stack([b1_s.reshape(HC, 128).T, b1_t.reshape(HC, 128).T],
                  axis=1).reshape(128, 2 * HC)
    b2 = np.stack([b2_s.reshape(FC, 128).T, b2_t.reshape(FC, 128).T],
                  axis=1).reshape(128, 2 * FC)
    return (np.ascontiguousarray(w1, dtype=np.float16),
            np.ascontiguousarray(w2, dtype=np.float16),
            np.ascontiguousarray(b1, dtype=np.float32),
            np.ascontiguousarray(b2, dtype=np.float32))


_NC = None


def kernel(z, W1_s, b1_s, W2_s, b2_s, W1_t, b1_t, W2_t, b2_t):
    global _NC
    z = np.asarray(z, dtype=np.float32)
    args = [np.asarray(x, dtype=np.float32)
            for x in (W1_s, b1_s, W2_s, b2_s, W1_t, b1_t, W2_t, b2_t)]
    w1, w2, b1, b2 = prep_weights(*args)

    aT = z[:, 0::2].T            # [256, B]
    bT = z[:, 1::2].T

    if _NC is None:
        _NC = build()

    in_maps = []
    for c in range(NCORES):
        sl = slice(c * R_FULL, (c + 1) * R_FULL)
        in_maps.append({
            "at": np.ascontiguousarray(aT[:, sl], dtype=np.float16),
            "bt": np.ascontiguousarray(bT[:, sl], dtype=np.float32),
            "w1": w1, "w2": w2, "b1": b1, "b2": b2,
        })
    res = bass_utils.run_bass_kernel_spmd(_NC, in_maps, core_ids=list(range(NCORES)))

    zo = np.empty((B, D), dtype=np.float32)
    zo[:, 0::2] = z[:, 0::2]
    for c in range(NCORES):
        zo[c * R_FULL:(c + 1) * R_FULL, 1::2] = res.results[c]["bo"].T
    ldv = np.concatenate([r["ld"] for r in res.results], axis=0)
    return zo, ldv


# revision 12
# speedup vs baseline: 1.2005x; 1.2005x over previous
"""Trainium2 Bass kernel for nn_AffineCoupling_884763263531.

Reference computation (B=65536, D=512, H=2048, half=256):
    a = z[:, 0::2]; b = z[:, 1::2]
    s = relu(a @ W1_s + b1_s) @ W2_s + b2_s
    t = relu(a @ W1_t + b1_t) @ W2_t + b2_t
    z_out = interleave(a, b * exp(s) + t);  logdet = s.sum(1)

Strategy: data-parallel over batch across 8 cores (8192 rows each).
The host hands each core feature-major (transposed) activations so both MLP
layers contract along the partition dim with zero on-device transposes:
  at [256, 8192] fp16  = a^T          (matmul operand)
  bt [256, 8192] fp32  = b^T          (exact passthrough for the epilogue)
Device: L1 fp16 matmuls -> PSUM -> fused bias+relu evac to fp16 h;
L2 fp16 matmuls -> s,t; logdet via ones-vector matmul over s^T partitions;
epilogue b_out^T = b^T * exp(s^T) + t^T, all feature-major.
Host re-interleaves: z_out[:,0::2]=a (exact), z_out[:,1::2]=b_out^T.T.
"""

import sys

for _p in ("/opt/trn_rl_repo",):
    if _p not in sys.path:
        sys.path.append(_p)

import numpy as np

import concourse.bass as bass
import concourse.mybir as mybir
from concourse import bacc
from concourse import bass_utils
from concourse.tile import TileContext

NCORES = 8
B, D, H = 65536, 512, 2048
HALF = D // 2           # 256
R_FULL = B // NCORES    # 8192 rows per core
BLK = 512               # rows per block
HC = H // 128           # 16 hidden chunks
FC = HALF // 128        # 2 output-feature chunks

F32 = mybir.dt.float32
F16 = mybir.dt.float16
AF = mybir.ActivationFunctionType
ALU = mybir.AluOpType


def build(rows=R_FULL, blk=BLK):
    # last 512 rows as two 256-row blocks: halves the non-overlapped tail
    blocks = []
    r0 = 0
    while r0 < rows:
        bsz = blk if r0 + blk < rows or rows <= blk else blk // 2
        bsz = min(bsz, rows - r0)
        blocks.append((r0, bsz))
        r0 += bsz
    nc = bacc.Bacc("TRN2", debug=False, num_devices=NCORES)

    at = nc.dram_tensor("at", [HALF, rows], F16, kind="ExternalInput")
    bt = nc.dram_tensor("bt", [HALF, rows], F32, kind="ExternalInput")
    w1 = nc.dram_tensor("w1", [128, 2 * 2 * H], F16, kind="ExternalInput")
    w2 = nc.dram_tensor("w2", [128, 2 * HC * HALF], F16, kind="ExternalInput")
    b1 = nc.dram_tensor("b1", [128, 2 * HC], F32, kind="ExternalInput")
    b2 = nc.dram_tensor("b2", [128, 2 * FC], F32, kind="ExternalInput")
    bo = nc.dram_tensor("bo", [HALF, rows], F32, kind="ExternalOutput")
    ld = nc.dram_tensor("ld", [rows], F32, kind="ExternalOutput")

    with TileContext(nc) as tc:
        with tc.tile_pool(name="const", bufs=1) as const, \
             tc.tile_pool(name="ap_", bufs=4) as ap_, \
             tc.tile_pool(name="bp", bufs=4) as bp, \
             tc.tile_pool(name="hp", bufs=3) as hp, \
             tc.tile_pool(name="stp", bufs=2) as stp, \
             tc.tile_pool(name="ep", bufs=4) as ep, \
             tc.tile_pool(name="tmpp", bufs=4) as tmpp, \
             tc.tile_pool(name="bop", bufs=4) as bop, \
             tc.tile_pool(name="ldp", bufs=2) as ldp, \
             tc.tile_pool(name="ps_h", bufs=4, space="PSUM") as ps_h, \
             tc.tile_pool(name="ps_l2", bufs=3, space="PSUM") as ps_l2, \
             tc.tile_pool(name="ps_ld", bufs=1, space="PSUM") as ps_ld:

            w1_sb = const.tile([128, 2 * 2 * H], F16)
            w2_sb = const.tile([128, 2 * HC * HALF], F16)
            b1_sb = const.tile([128, 2 * HC], F32)
            b2_sb = const.tile([128, 2 * FC], F32)
            ones = const.tile([128, 1], F16)

            nc.gpsimd.memset(ones[:], 1.0)

            # prefetch activations for the first blocks BEFORE the 8 MB of
            # weights so the first matmuls are not stuck behind them in the
            # DMA queue; weights themselves load in first-use order
            prefetched = {}
            for bi in range(min(2, len(blocks))):
                r0, bsz = blocks[bi]
                tiles = []
                for k in range(FC):
                    a_t = ap_.tile([128, bsz], F16, tag="at", name=f"pat_{bi}_{k}")
                    nc.sync.dma_start(a_t[:], at[k * 128:(k + 1) * 128, r0:r0 + bsz])
                    b_t = bp.tile([128, bsz], F32, tag="bt", name=f"pbt_{bi}_{k}")
                    nc.sync.dma_start(b_t[:], bt[k * 128:(k + 1) * 128, r0:r0 + bsz])
                    tiles.append((a_t, b_t))
                prefetched[bi] = tiles
                if bi == 0:
                    for q in range(2):
                        nc.sync.dma_start(w1_sb[:, q * H:(q + 1) * H],
                                          w1[:, q * H:(q + 1) * H])
                    nc.sync.dma_start(b1_sb[:], b1[:])
            for q in range(2, 4):
                nc.sync.dma_start(w1_sb[:, q * H:(q + 1) * H],
                                  w1[:, q * H:(q + 1) * H])
            for q in range(4):
                nc.sync.dma_start(w2_sb[:, q * 2048:(q + 1) * 2048],
                                  w2[:, q * 2048:(q + 1) * 2048])
            nc.sync.dma_start(b2_sb[:], b2[:])

            for bi, (r0, bsz) in enumerate(blocks):
                if bi in prefetched:
                    ats = [t[0] for t in prefetched[bi]]
                    bts = [t[1] for t in prefetched[bi]]
                else:
                    ats = []
                    bts = []
                    for k in range(FC):
                        a_t = ap_.tile([128, bsz], F16, tag="at", name=f"at_{bi}_{k}")
                        nc.sync.dma_start(a_t[:], at[k * 128:(k + 1) * 128, r0:r0 + bsz])
                        ats.append(a_t)
                        b_t = bp.tile([128, bsz], F32, tag="bt", name=f"bt_{bi}_{k}")
                        nc.sync.dma_start(b_t[:], bt[k * 128:(k + 1) * 128, r0:r0 + bsz])
                        bts.append(b_t)

                # ---- layer 1 (both MLPs): hT[m] [p=h-chunk, free=(hc, row)] fp16
                hts = []
                for m in range(2):
                    ht = hp.tile([128, HC * bsz], F16, tag="h", name=f"h_{bi}_{m}")
                    hts.append(ht)
                    for hc in range(HC):
                        ph = ps_h.tile([128, bsz], F32, tag="ph", name=f"ph_{bi}_{m}_{hc}")
                        for k in range(FC):
                            nc.tensor.matmul(
                                ph[:],
                                w1_sb[:, ((m * 2 + k) * H + hc * 128):((m * 2 + k) * H + hc * 128 + 128)],
                                ats[k][:],
                                start=(k == 0), stop=(k == FC - 1))
                        dst = ht[:, hc * bsz:(hc + 1) * bsz]
                        bias_ap = b1_sb[:, m * HC + hc: m * HC + hc + 1]
                        if hc % 2 == 0:
                            nc.scalar.activation(dst, ph[:], AF.Relu, bias=bias_ap)
                        else:
                            nc.vector.tensor_scalar(
                                dst, ph[:], bias_ap, 0.0, op0=ALU.add, op1=ALU.max)

                # ---- layer 2: s (fp16, feeds exp+logdet), t (fp32)
                st_sb = []
                for m in range(2):
                    st = stp.tile([128, FC * bsz], F16 if m == 0 else F32,
                                  tag=("s" if m == 0 else "t"), name=f"st_{bi}_{m}")
                    st_sb.append(st)
                    for mc in range(FC):
                        pl2 = ps_l2.tile([128, bsz], F32, tag="l2", name=f"pl2_{bi}_{m}_{mc}")
                        for hc in range(HC):
                            nc.tensor.matmul(
                                pl2[:],
                                w2_sb[:, ((m * HC + hc) * HALF + mc * 128):((m * HC + hc) * HALF + mc * 128 + 128)],
                                hts[m][:, hc * bsz:(hc + 1) * bsz],
                                start=(hc == 0), stop=(hc == HC - 1))
                        dst = st[:, mc * bsz:(mc + 1) * bsz]
                        bias_ap = b2_sb[:, m * FC + mc: m * FC + mc + 1]
                        if m == 0:
                            nc.vector.tensor_scalar_add(dst, pl2[:], bias_ap)
                        else:
                            nc.scalar.activation(dst, pl2[:], AF.Identity, bias=bias_ap)

                # ---- logdet: ones^T @ s^T accumulated over both feature chunks
                pld = ps_ld.tile([1, bsz], F32, tag="ld", name=f"pld_{bi}")
                for mc in range(FC):
                    nc.tensor.matmul(pld[:], ones[:],
                                     st_sb[0][:, mc * bsz:(mc + 1) * bsz],
                                     start=(mc == 0), stop=(mc == FC - 1))
                ld_sb = ldp.tile([1, bsz], F32, tag="lds", name=f"lds_{bi}")
                nc.vector.tensor_copy(ld_sb[:], pld[:])
                nc.sync.dma_start(ld[r0:r0 + bsz], ld_sb[:])

                # ---- epilogue (feature-major): bo = bt * exp(s) + t
                for mc in range(FC):
                    sl = slice(mc * bsz, (mc + 1) * bsz)
                    ex = ep.tile([128, bsz], F32, tag="e", name=f"e_{bi}_{mc}")
                    nc.scalar.activation(ex[:], st_sb[0][:, sl], AF.Exp)
                    tmp = tmpp.tile([128, bsz], F32, tag="tm", name=f"tm_{bi}_{mc}")
                    nc.vector.tensor_mul(tmp[:], ex[:], bts[mc][:])
                    bo_sb = bop.tile([128, bsz], F32, tag="bo", name=f"bo_{bi}_{mc}")
                    nc.vector.tensor_add(bo_sb[:], tmp[:], st_sb[1][:, sl])
                    nc.sync.dma_start(bo[mc * 128:(mc + 1) * 128, r0:r0 + bsz], bo_sb[:])

    nc.compile()
    return nc


def prep_weights(W1_s, b1_s, W2_s, b2_s, W1_t, b1_t, W2_t, b2_t):
    def w1c(w):
        return w.reshape(2, 128, H).transpose(1, 0, 2)

    def w2c(w):
        return w.reshape(HC, 128, HALF).transpose(1, 0, 2)

    w1 = np.stack([w1c(W1_s), w1c(W1_t)], axis=1).reshape(128, 2 * 2 * H)
    w2 = np.stack([w2c(W2_s), w2c(W2_t)], axis=1).reshape(128, 2 * HC * HALF)
    b1 = np.stack([b1_s.reshape(HC, 128).T, b1_t.reshape(HC, 128).T],
                  axis=1).reshape(128, 2 * HC)
    b2 = np.stack([b2_s.reshape(FC, 128).T, b2_t.reshape(FC, 128).T],
                  axis=1).reshape(128, 2 * FC)
    return (np.ascontiguousarray(w1, dtype=np.float16),
            np.ascontiguousarray(w2, dtype=np.float16),
            np.ascontiguousarray(b1, dtype=np.float32),
            np.ascontiguousarray(b2, dtype=np.float32))


_NC = None


def kernel(z, W1_s, b1_s, W2_s, b2_s, W1_t, b1_t, W2_t, b2_t):
    global _NC
    z = np.asarray(z, dtype=np.float32)
    args = [np.asarray(x, dtype=np.float32)
            for x in (W1_s, b1_s, W2_s, b2_s, W1_t, b1_t, W2_t, b2_t)]
    w1, w2, b1, b2 = prep_weights(*args)

    aT = z[:, 0::2].T            # [256, B]
    bT = z[:, 1::2].T

    if _NC is None:
        _NC = build()

    in_maps = []
    for c in range(NCORES):
        sl = slice(c * R_FULL, (c + 1) * R_FULL)
        in_maps.append({
            "at": np.ascontiguousarray(aT[:, sl], dtype=np.float16),
            "bt": np.ascontiguousarray(bT[:, sl], dtype=np.float32),
            "w1": w1, "w2": w2, "b1": b1, "b2": b2,
        })
    res = bass_utils.run_bass_kernel_spmd(_NC, in_maps, core_ids=list(range(NCORES)))

    zo = np.empty((B, D), dtype=np.float32)
    zo[:, 0::2] = z[:, 0::2]
    for c in range(NCORES):
        zo[c * R_FULL:(c + 1) * R_FULL, 1::2] = res.results[c]["bo"].T
    ldv = np.concatenate([r["ld"] for r in res.results], axis=0)
    return zo, ldv


# revision 13
# speedup vs baseline: 1.2026x; 1.0017x over previous
"""Trainium2 Bass kernel for nn_AffineCoupling_884763263531.

Reference computation (B=65536, D=512, H=2048, half=256):
    a = z[:, 0::2]; b = z[:, 1::2]
    s = relu(a @ W1_s + b1_s) @ W2_s + b2_s
    t = relu(a @ W1_t + b1_t) @ W2_t + b2_t
    z_out = interleave(a, b * exp(s) + t);  logdet = s.sum(1)

Strategy: data-parallel over batch across 8 cores (8192 rows each).
The host hands each core feature-major (transposed) activations so both MLP
layers contract along the partition dim with zero on-device transposes:
  at [256, 8192] fp16  = a^T          (matmul operand)
  bt [256, 8192] fp32  = b^T          (exact passthrough for the epilogue)
Device: L1 fp16 matmuls -> PSUM -> fused bias+relu evac to fp16 h;
L2 fp16 matmuls -> s,t; logdet via ones-vector matmul over s^T partitions;
epilogue b_out^T = b^T * exp(s^T) + t^T, all feature-major.
Host re-interleaves: z_out[:,0::2]=a (exact), z_out[:,1::2]=b_out^T.T.
"""

import sys

for _p in ("/opt/trn_rl_repo",):
    if _p not in sys.path:
        sys.path.append(_p)

import numpy as np

import concourse.bass as bass
import concourse.mybir as mybir
from concourse import bacc
from concourse import bass_utils
from concourse.tile import TileContext

NCORES = 8
B, D, H = 65536, 512, 2048
HALF = D // 2           # 256
R_FULL = B // NCORES    # 8192 rows per core
BLK = 512               # rows per block
HC = H // 128           # 16 hidden chunks
FC = HALF // 128        # 2 output-feature chunks

F32 = mybir.dt.float32
F16 = mybir.dt.float16
AF = mybir.ActivationFunctionType
ALU = mybir.AluOpType


def build(rows=R_FULL, blk=BLK):
    # last 512 rows as two 256-row blocks: halves the non-overlapped tail
    blocks = []
    r0 = 0
    while r0 < rows:
        bsz = blk if r0 + blk < rows or rows <= blk else blk // 2
        bsz = min(bsz, rows - r0)
        blocks.append((r0, bsz))
        r0 += bsz
    nc = bacc.Bacc("TRN2", debug=False, num_devices=NCORES)

    at = nc.dram_tensor("at", [HALF, rows], F16, kind="ExternalInput")
    bt = nc.dram_tensor("bt", [HALF, rows], F32, kind="ExternalInput")
    w1 = nc.dram_tensor("w1", [128, 2 * 2 * H], F16, kind="ExternalInput")
    w2 = nc.dram_tensor("w2", [128, 2 * HC * HALF], F16, kind="ExternalInput")
    b1 = nc.dram_tensor("b1", [128, 2 * HC], F32, kind="ExternalInput")
    b2 = nc.dram_tensor("b2", [128, 2 * FC], F32, kind="ExternalInput")
    bo = nc.dram_tensor("bo", [HALF, rows], F32, kind="ExternalOutput")
    ld = nc.dram_tensor("ld", [rows], F32, kind="ExternalOutput")

    with TileContext(nc) as tc:
        with tc.tile_pool(name="const", bufs=1) as const, \
             tc.tile_pool(name="ap_", bufs=4) as ap_, \
             tc.tile_pool(name="bp", bufs=4) as bp, \
             tc.tile_pool(name="hp", bufs=3) as hp, \
             tc.tile_pool(name="stp", bufs=2) as stp, \
             tc.tile_pool(name="ep", bufs=4) as ep, \
             tc.tile_pool(name="tmpp", bufs=4) as tmpp, \
             tc.tile_pool(name="bop", bufs=4) as bop, \
             tc.tile_pool(name="ldp", bufs=2) as ldp, \
             tc.tile_pool(name="ps_h", bufs=4, space="PSUM") as ps_h, \
             tc.tile_pool(name="ps_l2", bufs=3, space="PSUM") as ps_l2, \
             tc.tile_pool(name="ps_ld", bufs=1, space="PSUM") as ps_ld:

            w1_sb = const.tile([128, 2 * 2 * H], F16)
            w2_sb = const.tile([128, 2 * HC * HALF], F16)
            b1_sb = const.tile([128, 2 * HC], F32)
            b2_sb = const.tile([128, 2 * FC], F32)
            ones = const.tile([128, 1], F16)

            nc.gpsimd.memset(ones[:], 1.0)

            # DMA emission in first-use order on the sync HWDGE queue:
            # block-0/1 `a` tiles, first-needed w1 slivers, then the rest.
            prefetched = {}
            pre_a = {}
            for bi in range(min(2, len(blocks))):
                r0, bsz = blocks[bi]
                pa = []
                for k in range(FC):
                    a_t = ap_.tile([128, bsz], F16, tag="at", name=f"pat_{bi}_{k}")
                    nc.sync.dma_start(a_t[:], at[k * 128:(k + 1) * 128, r0:r0 + bsz])
                    pa.append(a_t)
                pre_a[bi] = pa
                if bi == 0:
                    # w1 m=0 slivers, 4 hc-chunks (512 cols) at a time per k
                    for p in range(4):
                        for k in range(2):
                            off = k * H + p * 512
                            nc.sync.dma_start(w1_sb[:, off:off + 512],
                                              w1[:, off:off + 512])
                    nc.sync.dma_start(b1_sb[:], b1[:])
            for bi in range(min(2, len(blocks))):
                r0, bsz = blocks[bi]
                pb = []
                for k in range(FC):
                    b_t = bp.tile([128, bsz], F32, tag="bt", name=f"pbt_{bi}_{k}")
                    nc.sync.dma_start(b_t[:], bt[k * 128:(k + 1) * 128, r0:r0 + bsz])
                    pb.append(b_t)
                prefetched[bi] = list(zip(pre_a[bi], pb))
            for p in range(4):
                for k in range(2):
                    off = (2 + k) * H + p * 512
                    nc.sync.dma_start(w1_sb[:, off:off + 512], w1[:, off:off + 512])
            for q in range(4):
                nc.sync.dma_start(w2_sb[:, q * 2048:(q + 1) * 2048],
                                  w2[:, q * 2048:(q + 1) * 2048])
            nc.sync.dma_start(b2_sb[:], b2[:])

            for bi, (r0, bsz) in enumerate(blocks):
                if bi in prefetched:
                    ats = [t[0] for t in prefetched[bi]]
                    bts = [t[1] for t in prefetched[bi]]
                else:
                    ats = []
                    bts = []
                    for k in range(FC):
                        a_t = ap_.tile([128, bsz], F16, tag="at", name=f"at_{bi}_{k}")
                        nc.sync.dma_start(a_t[:], at[k * 128:(k + 1) * 128, r0:r0 + bsz])
                        ats.append(a_t)
                        b_t = bp.tile([128, bsz], F32, tag="bt", name=f"bt_{bi}_{k}")
                        nc.sync.dma_start(b_t[:], bt[k * 128:(k + 1) * 128, r0:r0 + bsz])
                        bts.append(b_t)

                # ---- layer 1 (both MLPs): hT[m] [p=h-chunk, free=(hc, row)] fp16
                hts = []
                for m in range(2):
                    ht = hp.tile([128, HC * bsz], F16, tag="h", name=f"h_{bi}_{m}")
                    hts.append(ht)
                    for hc in range(HC):
                        ph = ps_h.tile([128, bsz], F32, tag="ph", name=f"ph_{bi}_{m}_{hc}")
                        for k in range(FC):
                            nc.tensor.matmul(
                                ph[:],
                                w1_sb[:, ((m * 2 + k) * H + hc * 128):((m * 2 + k) * H + hc * 128 + 128)],
                                ats[k][:],
                                start=(k == 0), stop=(k == FC - 1))
                        dst = ht[:, hc * bsz:(hc + 1) * bsz]
                        bias_ap = b1_sb[:, m * HC + hc: m * HC + hc + 1]
                        if hc % 2 == 0:
                            nc.scalar.activation(dst, ph[:], AF.Relu, bias=bias_ap)
                        else:
                            nc.vector.tensor_scalar(
                                dst, ph[:], bias_ap, 0.0, op0=ALU.add, op1=ALU.max)

                # ---- layer 2: s (fp16, feeds exp+logdet), t (fp32)
                st_sb = []
                for m in range(2):
                    st = stp.tile([128, FC * bsz], F16 if m == 0 else F32,
                                  tag=("s" if m == 0 else "t"), name=f"st_{bi}_{m}")
                    st_sb.append(st)
                    for mc in range(FC):
                        pl2 = ps_l2.tile([128, bsz], F32, tag="l2", name=f"pl2_{bi}_{m}_{mc}")
                        for hc in range(HC):
                            nc.tensor.matmul(
                                pl2[:],
                                w2_sb[:, ((m * HC + hc) * HALF + mc * 128):((m * HC + hc) * HALF + mc * 128 + 128)],
                                hts[m][:, hc * bsz:(hc + 1) * bsz],
                                start=(hc == 0), stop=(hc == HC - 1))
                        dst = st[:, mc * bsz:(mc + 1) * bsz]
                        bias_ap = b2_sb[:, m * FC + mc: m * FC + mc + 1]
                        if m == 0:
                            nc.vector.tensor_scalar_add(dst, pl2[:], bias_ap)
                        else:
                            nc.scalar.activation(dst, pl2[:], AF.Identity, bias=bias_ap)

                # ---- logdet: ones^T @ s^T accumulated over both feature chunks
                pld = ps_ld.tile([1, bsz], F32, tag="ld", name=f"pld_{bi}")
                for mc in range(FC):
                    nc.tensor.matmul(pld[:], ones[:],
                                     st_sb[0][:, mc * bsz:(mc + 1) * bsz],
                                     start=(mc == 0), stop=(mc == FC - 1))
                ld_sb = ldp.tile([1, bsz], F32, tag="lds", name=f"lds_{bi}")
                nc.vector.tensor_copy(ld_sb[:], pld[:])
                nc.sync.dma_start(ld[r0:r0 + bsz], ld_sb[:])

                # ---- epilogue (feature-major): bo = bt * exp(s) + t
                for mc in range(FC):
                    sl = slice(mc * bsz, (mc + 1) * bsz)
                    ex = ep.tile([128, bsz], F32, tag="e", name=f"e_{bi}_{mc}")
                    nc.scalar.activation(ex[:], st_sb[0][:, sl], AF.Exp)
                    tmp = tmpp.tile([128, bsz], F32, tag="tm", name=f"tm_{bi}_{mc}")
                    nc.vector.tensor_mul(tmp[:], ex[:], bts[mc][:])
                    bo_sb = bop.tile([128, bsz], F32, tag="bo", name=f"bo_{bi}_{mc}")
                    nc.vector.tensor_add(bo_sb[:], tmp[:], st_sb[1][:, sl])
                    nc.sync.dma_start(bo[mc * 128:(mc + 1) * 128, r0:r0 + bsz], bo_sb[:])

    nc.compile()
    return nc


def prep_weights(W1_s, b1_s, W2_s, b2_s, W1_t, b1_t, W2_t, b2_t):
    def w1c(w):
        return w.reshape(2, 128, H).transpose(1, 0, 2)

    def w2c(w):
        return w.reshape(HC, 128, HALF).transpose(1, 0, 2)

    w1 = np.stack([w1c(W1_s), w1c(W1_t)], axis=1).reshape(128, 2 * 2 * H)
    w2 = np.stack([w2c(W2_s), w2c(W2_t)], axis=1).reshape(128, 2 * HC * HALF)
    b1 = np.stack([b1_s.reshape(HC, 128).T, b1_t.reshape(HC, 128).T],
                  axis=1).reshape(128, 2 * HC)
    b2 = np.stack([b2_s.reshape(FC, 128).T, b2_t.reshape(FC, 128).T],
                  axis=1).reshape(128, 2 * FC)
    return (np.ascontiguousarray(w1, dtype=np.float16),
            np.ascontiguousarray(w2, dtype=np.float16),
            np.ascontiguousarray(b1, dtype=np.float32),
            np.ascontiguousarray(b2, dtype=np.float32))


_NC = None


def kernel(z, W1_s, b1_s, W2_s, b2_s, W1_t, b1_t, W2_t, b2_t):
    global _NC
    z = np.asarray(z, dtype=np.float32)
    args = [np.asarray(x, dtype=np.float32)
            for x in (W1_s, b1_s, W2_s, b2_s, W1_t, b1_t, W2_t, b2_t)]
    w1, w2, b1, b2 = prep_weights(*args)

    aT = z[:, 0::2].T            # [256, B]
    bT = z[:, 1::2].T

    if _NC is None:
        _NC = build()

    in_maps = []
    for c in range(NCORES):
        sl = slice(c * R_FULL, (c + 1) * R_FULL)
        in_maps.append({
            "at": np.ascontiguousarray(aT[:, sl], dtype=np.float16),
            "bt": np.ascontiguousarray(bT[:, sl], dtype=np.float32),
            "w1": w1, "w2": w2, "b1": b1, "b2": b2,
        })
    res = bass_utils.run_bass_kernel_spmd(_NC, in_maps, core_ids=list(range(NCORES)))

    zo = np.empty((B, D), dtype=np.float32)
    zo[:, 0::2] = z[:, 0::2]
    for c in range(NCORES):
        zo[c * R_FULL:(c + 1) * R_FULL, 1::2] = res.results[c]["bo"].T
    ldv = np.concatenate([r["ld"] for r in res.results], axis=0)
    return zo, ldv
